# revision 22
# baseline (speedup 1.0000x reference)
"""Trainium2 Bass kernel for nn_MixedLinear_KV (moe_routing, memory-bound).

Math: the reference computes
    x_mix = sum_m coef_a[m] * fake_quant(x, a_scales[m], AB[m])
    w_mix = sum_{i,j,n} coef_w[i,j,n] * fake_quant(pad_ij(W), w_scales[n], WB[n])
    b_mix = sum_{i,j} coef_b[i,j] * pad_ij(b)
    out   = x_mix @ w_mix.T + b_mix

With the benchmark inputs (a_scales == 1, x ~ N(0,1) so |x| < 7.5 always,
verified at runtime), both activation fake-quants reduce to rint(x), so
    out = rint(x) @ (s * w_mix).T + b_mix,   s = coef_a.sum()

Device strategy (all-fp8 DoubleRow):
  - q = rint(x) are small integers, exact in fp8 e4m3. Host packs q as
    fp8 (4 MiB/core instead of 16 MiB fp32) laid out k-major.
  - Weights W = s*w_mix are scaled by 2^sexp (max ~224) and quantized to
    e4m3 (W8). The residual E = Ws - W8 is ALSO e4m3-quantized (E8) at
    the same scale basis (fp8 has ~18 octaves of range, plenty).
  - The PE runs DoubleRow fp8 matmuls (2 weights/cell, K=256/matmul,
    2x bf16 rate): 4 main matmuls (all 8 k-chunks in pairs) plus 2
    correction matmuls (E8 for k<512, where the mixture places ~83% of
    the weight energy; k>=512 has little energy because only the larger
    hidden_size configs reach it). Measured end-to-end rel err 0.0094
    vs the 2e-2 gate.
  - DVE adds the (pre-scaled) bias during PSUM->SBUF, stores fp16;
    host divides by 2^sexp when widening to fp32 (exact, power of 2).

Sharding: data-parallel over the batch dim (8 batches -> 8 cores).
"""

import sys

sys.path.insert(0, "/opt/trn_rl_repo")

import json
import math

import ml_dtypes
import numpy as np

import concourse.bass as bass
import concourse.mybir as mybir
from concourse import tile
from concourse.bass_utils import run_bass_kernel_spmd

# Problem constants (hardcoded per task contract)
B, S, D_IN, D_OUT = 8, 4096, 1024, 512
HS = [512, 768, 1024]
NH = [8, 12, 16]
NKV = 4
AB = [4, 8]
WB = [4, 8]
N_CORES = 8
N_TB = 8  # token blocks of 512
N_CH = 8  # k chunks of 128
F8 = ml_dtypes.float8_e4m3fn


def _split_multi_waits(bir_bytes: bytes) -> bytes:
    """This container's walrus supports only one sem-wait per instruction;
    hoist extra waits onto preceding NoOps on the same engine."""
    bir = json.loads(bir_bytes)
    for fn in bir["functions"]:
        for bb in fn["blocks"]:
            new_insts = []
            for inst in bb["instructions"]:
                si = inst.get("sync_info") or {}
                ow = si.get("on_wait") or []
                if len(ow) > 1:
                    for k, w in enumerate(ow[:-1]):
                        new_insts.append(
                            {
                                "debug": inst.get("debug", 0),
                                "engine": inst["engine"],
                                "ins": [],
                                "outs": [],
                                "name": f"{inst['name']}_wsplit{k}",
                                "opcode": "NoOp",
                                "sync_info": {"on_wait": [w]},
                            }
                        )
                    si["on_wait"] = [ow[-1]]
                new_insts.append(inst)
            bb["instructions"] = new_insts
    return json.dumps(bir).encode()


def _host_fold_weights(weight, bias, mix_weights, a_scales, w_scales):
    """Mirror the reference's fp32 weight mixture exactly; return
    (w_eff [512,1024] f32 = s*w_mix, b_mix [512] f32, w_mix [512,1024])."""
    w32 = np.asarray(weight, np.float32)
    b32 = np.asarray(bias, np.float32)
    mw = np.asarray(mix_weights, np.float32).reshape(3, 3, 2, 2)
    w_sc = np.asarray(w_scales, np.float32)

    coef_a = mw.sum(axis=(0, 1, 3))  # [2]
    coef_w = mw.sum(axis=2)  # [3,3,2]
    coef_b = mw.sum(axis=(2, 3))  # [3,3]

    w_mix = np.zeros((D_OUT, D_IN), np.float32)
    b_mix = np.zeros((D_OUT,), np.float32)
    for i, h in enumerate(HS):
        for j, nh in enumerate(NH):
            out_dim = NKV * (h // nh)
            w_pad = np.zeros((D_OUT, D_IN), np.float32)
            w_pad[:out_dim, :h] = w32[:out_dim, :h]
            b_pad = np.zeros((D_OUT,), np.float32)
            b_pad[:out_dim] = b32[:out_dim]
            for n, wb in enumerate(WB):
                qn, qp = -(2 ** (wb - 1)), 2 ** (wb - 1) - 1
                xs = w_pad / w_sc[n]
                xc = np.clip(xs, np.float32(qn), np.float32(qp))
                fq = np.rint(xc) * w_sc[n]
                w_mix = w_mix + coef_w[i, j, n] * fq
            b_mix = b_mix + coef_b[i, j] * b_pad

    s = np.float64(coef_a[0]) + np.float64(coef_a[1])
    w_eff = (s * w_mix.astype(np.float64)).astype(np.float32)  # [512, 1024]
    return w_eff, b_mix, w_mix


def _quantize_weights(w_eff, b_mix):
    """fp8 main + residual weights. Returns (wf [128,6,2,512] f8,
    brep [128,512] f32, w_dev [512,1024] f32, sexp)."""
    amax = float(np.abs(w_eff).max())
    if amax <= 0.0 or not np.isfinite(amax):
        sexp = 0
    else:
        sexp = int(math.floor(math.log2(224.0 / amax)))
        sexp = max(min(sexp, 30), -30)
    sc = np.float32(2.0**sexp)
    ws = (w_eff * sc).astype(np.float32)
    w8 = ws.astype(F8).astype(np.float32)
    e8 = (ws - w8).astype(F8).astype(np.float32)

    # wf[p, s, i, o], set order chosen so a prefix DMA covers the first
    # matmuls of each group: [W8 pair0, E8 pair0, W8 pair1, E8 pair1,
    # W8 pair2, W8 pair3]  (true k = (2*pair+i)*128 + p; E8 only k<512)
    w8k = w8.T.reshape(4, 2, 128, D_OUT)  # [pair, i, p, o]
    e8k = e8.T.reshape(4, 2, 128, D_OUT)
    wf = np.empty((128, 6, 2, D_OUT), np.float32)
    wf[:, 0] = w8k[0].transpose(1, 0, 2)
    wf[:, 1] = e8k[0].transpose(1, 0, 2)
    wf[:, 2] = w8k[1].transpose(1, 0, 2)
    wf[:, 3] = e8k[1].transpose(1, 0, 2)
    wf[:, 4] = w8k[2].transpose(1, 0, 2)
    wf[:, 5] = w8k[3].transpose(1, 0, 2)
    wf = np.ascontiguousarray(wf).astype(F8)

    corr = np.zeros_like(w8)
    corr[:, :512] = e8[:, :512]
    w_dev = ((w8 + corr) / sc).astype(np.float32)  # effective device weights

    # bias replicated over partitions; fp16 is plenty (|bias*sc| ~ 1e2)
    brep = np.ascontiguousarray(
        np.broadcast_to((b_mix * sc).astype(np.float16), (128, D_OUT))
    )
    return wf, brep, w_dev, sexp


def _pack_q(xb):
    """xb [4096, 1024] f32 -> q8 [128, 8tb, 8c, 512t] f8e4m3 with
    q8[p, tb, c, t] = e4m3(clip(rint(xb[tb*512+t, c*128+p])))."""
    q = np.rint(xb)
    np.clip(q, -240.0, 240.0, out=q)
    q8 = q.astype(F8)  # cast while t-major (cheap)
    # [4096, 1024] -> [8tb, 512t, 8c, 128p] -> [p, tb, c, t]
    return np.ascontiguousarray(
        q8.reshape(N_TB, 512, N_CH, 128).transpose(3, 0, 2, 1)
    )


def _build_nc():
    f32, f16, f8 = mybir.dt.float32, mybir.dt.float16, mybir.dt.float8e4
    DR = mybir.MatmulPerfMode.DoubleRow
    nc = bass.Bass("TRN2", target_bir_lowering=False, debug=False)

    q8_d = nc.dram_tensor("q8", [128, N_TB, N_CH, 512], f8, kind="ExternalInput").ap()
    wf_d = nc.dram_tensor("wf", [128, 6, 2, D_OUT], f8, kind="ExternalInput").ap()
    br_d = nc.dram_tensor("brep", [128, D_OUT], f16, kind="ExternalInput").ap()
    out_d = nc.dram_tensor("out", [S, D_OUT], f16, kind="ExternalOutput").ap()

    with tile.TileContext(nc) as tc:
        with (
            tc.tile_pool(name="const", bufs=1) as cpool,
            tc.tile_pool(name="qlo", bufs=3) as lopool,
            tc.tile_pool(name="qhi", bufs=3) as hipool,
            tc.tile_pool(name="op", bufs=6) as opool,
            tc.tile_pool(name="ps", bufs=4, space="PSUM") as pspool,
        ):
            # PE prewarm: full-width matmuls on a memset tile keep the HAM
            # activity monitor seeing a BUSY array (N=16 dummies only hit
            # ~25% occupancy and leave the clock throttled) while the NEFF
            # startup + first input DMAs complete, so the real stream runs
            # at 2.4 GHz from its first issue.
            z_sb = cpool.tile([128, D_OUT], f8)
            nc.vector.memset(z_sb[:], 0.0)
            # scratch psum from the main pool: every real group begins with
            # start=True (overwrite semantics), so dummy garbage is harmless
            zp = pspool.tile([128, 2, D_OUT], f32, tag="ps")
            N_WARM = 11
            for i in range(N_WARM):
                nc.tensor.matmul(
                    zp[:, 0, :],
                    lhsT=z_sb[:, :128],
                    rhs=z_sb[:, :],
                    start=(i == 0),
                    stop=(i == N_WARM - 1),
                )

            # first ~1.3 MB (weights + block 0) spread over all three DMA
            # paths in just-in-time order, so the early matmul stream isn't
            # gated on a single ring's ramp-up
            wf_sb = cpool.tile([128, 6, 2, D_OUT], f8)
            nc.scalar.dma_start(out=wf_sb[:, 0:2], in_=wf_d[:, 0:2])
            nc.gpsimd.dma_start(out=wf_sb[:, 4:6], in_=wf_d[:, 4:6])
            br_sb = cpool.tile([128, D_OUT], f16)

            # (wf set, q chunk): sets 0-3 = {W8,E8} pairs 0-1 (k<512, qlo),
            # sets 4-5 = W8 pairs 2-3 (qhi)
            MM_PLAN = [(0, 0), (1, 0), (2, 2), (3, 2), (4, 4), (5, 6)]

            def emit_load(tb):
                qlo = lopool.tile([128, 4, 512], f8, tag="qlo")
                nc.sync.dma_start(out=qlo[:], in_=q8_d[:, tb, 0:4])
                if tb == 0:
                    # block 0's high half split across the sync ring and the
                    # (otherwise idle at start) SWDGE path; weight sets 2-3
                    # ride sync between them, just in time for matmuls 5-8
                    nc.sync.dma_start(out=wf_sb[:, 2:4], in_=wf_d[:, 2:4])
                    hia = hipool.tile([128, 2, 512], f8, tag="qhia")
                    nc.sync.dma_start(out=hia[:], in_=q8_d[:, tb, 4:6])
                    hib = hipool.tile([128, 2, 512], f8, tag="qhib")
                    nc.gpsimd.dma_start(out=hib[:], in_=q8_d[:, tb, 6:8])
                    nc.gpsimd.dma_start(out=br_sb[:], in_=br_d[:])
                    return qlo, (hia, hib)
                qhi = hipool.tile([128, 4, 512], f8, tag="qhi")
                nc.scalar.dma_start(out=qhi[:], in_=q8_d[:, tb, 4:8])
                return qlo, qhi

            def q_slice(qlo, qhi, c):
                if c < 4:
                    return qlo, c
                if isinstance(qhi, tuple):
                    return qhi[(c - 4) // 2], 0
                return qhi, c - 4

            def emit_compute(tb, qlo, qhi):
                final = tb == N_TB - 1
                for h in range(2):
                    t0 = tb * 512 + h * 256
                    if final and h == 1:
                        # last half-block: two single-slice groups so the
                        # first drains while the second still computes, and
                        # half-width drains/stores shorten the very tail
                        for ts in range(2):
                            tok = (2 * h + ts) * 128
                            ps = pspool.tile([128, 2, D_OUT], f32, tag="ps")
                            for si, (s, c) in enumerate(MM_PLAN):
                                q_sb, cc = q_slice(qlo, qhi, c)
                                nc.tensor.matmul(
                                    ps[:, 0, :],
                                    lhsT=q_sb[:, cc : cc + 2, tok : tok + 128],
                                    rhs=wf_sb[:, s, :, :],
                                    start=(si == 0),
                                    stop=(si == len(MM_PLAN) - 1),
                                    perf_mode=DR,
                                )
                            st_eng = nc.sync if ts == 0 else nc.scalar
                            for half in range(2):
                                o_sb = opool.tile([128, D_OUT // 2], f16, tag="oh")
                                nc.vector.tensor_add(
                                    o_sb[:],
                                    ps[:, 0, half * 256 : half * 256 + 256],
                                    br_sb[:, half * 256 : half * 256 + 256],
                                )
                                st_eng.dma_start(
                                    out=out_d[
                                        t0 + ts * 128 : t0 + ts * 128 + 128,
                                        half * 256 : half * 256 + 256,
                                    ],
                                    in_=o_sb[:],
                                )
                        continue
                    ps = pspool.tile([128, 2, D_OUT], f32, tag="ps")
                    for si, (s, c) in enumerate(MM_PLAN):
                        q_sb, cc = q_slice(qlo, qhi, c)
                        for ts in range(2):
                            tok = (2 * h + ts) * 128
                            nc.tensor.matmul(
                                ps[:, ts, :],
                                lhsT=q_sb[:, cc : cc + 2, tok : tok + 128],
                                rhs=wf_sb[:, s, :, :],
                                start=(si == 0),
                                stop=(si == len(MM_PLAN) - 1),
                                perf_mode=DR,
                            )
                    # per-slice drains + stores: shorter critical tail; the
                    # last block's stores ride the (by then idle) HW rings
                    for ts in range(2):
                        o_sb = opool.tile([128, D_OUT], f16, tag="o")
                        nc.vector.tensor_add(o_sb[:], ps[:, ts, :], br_sb[:])
                        st_eng = nc.sync if final else nc.gpsimd
                        if final and ts == 1:
                            st_eng = nc.scalar
                        st_eng.dma_start(
                            out=out_d[t0 + ts * 128 : t0 + ts * 128 + 128, :],
                            in_=o_sb[:],
                        )

            pending = []
            for tb in range(N_TB):
                pending.append((tb, *emit_load(tb)))
                if len(pending) > 1:
                    emit_compute(*pending.pop(0))
            for args in pending:
                emit_compute(*args)

    orig = nc.to_json_bytes
    nc.to_json_bytes = lambda: _split_multi_waits(orig())
    return nc


_NC_CACHE = None


def _fq32(x, scale, bits):
    """fp32 fake_quant forward value, matching the reference bitwise."""
    qn, qp = -(2 ** (bits - 1)), 2 ** (bits - 1) - 1
    xs = (np.asarray(x, np.float32) / np.float32(scale)).astype(np.float32)
    xc = np.clip(xs, np.float32(qn), np.float32(qp))
    return (np.rint(xc) * np.float32(scale)).astype(np.float32)


def _x_mix_ref(x, mix_weights, a_scales):
    """The reference's activation mixture, in fp32."""
    mw = np.asarray(mix_weights, np.float32).reshape(3, 3, 2, 2)
    coef_a = mw.sum(axis=(0, 1, 3))
    xm = coef_a[0] * _fq32(x, a_scales[0], AB[0])
    return (xm + coef_a[1] * _fq32(x, a_scales[1], AB[1])).astype(np.float32)


def kernel(x, weight, bias, mix_weights, a_scales, w_scales):
    global _NC_CACHE
    x = np.asarray(x, np.float32)
    assert x.shape == (B, S, D_IN)
    a_sc = np.asarray(a_scales, np.float32)

    w_eff, b_mix, w_mix = _host_fold_weights(
        weight, bias, mix_weights, a_scales, w_scales
    )

    if not np.all(a_sc == np.float32(1.0)):
        # General-scale fallback (benchmark inputs always have a_scales == 1):
        # compute the reference mixture on host in fp32.
        x_mix = _x_mix_ref(x, mix_weights, a_scales)
        return (np.einsum("bsi,oi->bso", x_mix, w_mix) + b_mix).astype(np.float32)

    wf, brep, w_dev, sexp = _quantize_weights(w_eff, b_mix)
    inv_sc = np.float32(2.0**-sexp)

    if _NC_CACHE is None:
        _NC_CACHE = _build_nc()
    nc = _NC_CACHE

    in_maps = [
        {"q8": _pack_q(x[b]), "wf": wf, "brep": brep} for b in range(N_CORES)
    ]
    try:
        res = run_bass_kernel_spmd(nc, in_maps, list(range(N_CORES)))
    except Exception:
        # one retry for transient device errors
        res = run_bass_kernel_spmd(nc, in_maps, list(range(N_CORES)))
    out = np.stack(
        [res.results[b]["out"].astype(np.float32) for b in range(N_CORES)], axis=0
    )
    out *= inv_sc

    if np.isinf(out).any() or np.isnan(out).any():
        # fp16 overflow guard for pathological inputs: full host fallback
        # (never triggers for the benchmark distribution).
        x_mix = _x_mix_ref(x, mix_weights, a_scales)
        return (np.einsum("bsi,oi->bso", x_mix, w_mix) + b_mix).astype(np.float32)

    # Exact host patch for |x| >= 7.49, where rint(x) differs from the
    # reference's clipped fake-quants (x ~ N(0,1) in the benchmark: never
    # triggers; keeps kernel() correct for arbitrary inputs).
    idx = np.argwhere(np.abs(x) >= 7.49)
    if len(idx):
        for b, t, i in idx:
            xv = x[b, t, i]
            ref_xmix = _x_mix_ref(xv, mix_weights, a_sc)
            # what the device computed for this element (same IEEE ops)
            dev_q = np.float32(
                np.clip(np.rint(np.float32(xv)), -240.0, 240.0).astype(F8)
            )
            out[b, t, :] += ref_xmix * w_mix[:, i] - dev_q * w_dev[:, i]
    return out
